# revision 14
# baseline (speedup 1.0000x reference)
"""Trainium2 Bass kernel for EntityAwareLSTMLayer.

Problem (hardcoded):
  B=1024, T=365, DYN=32, STATIC=27, UNITS=256
  i_gate = sigmoid(x_static @ W_sh + bias_s)            [B, U]   (static, once)
  gx_t   = x_t @ W_ih + bias                            [B, 3U]
  gates  = gx_t + h @ W_hh                              [B, 3U]  (f|o|g)
  c      = sigmoid(f) * c + i_gate * tanh(g)
  h      = sigmoid(o) * tanh(c)
  return h_final                                        [B, U]

Sharding: data-parallel over batch, 8 cores x 128 rows. The per-step
recurrence is a long serial chain of small cross-engine ops (PE matmul ->
ACT sigmoid/tanh -> DVE mul/add -> ACT tanh -> DVE mul -> PE transpose),
dominated by fixed per-instruction access latencies, so a single 128-row
stream leaves every engine mostly idle. The 128 rows are therefore split
into NP=2 independent 64-row "pipes" run half a step out of phase:
engine costs are charged on the free dimension only, so each pipe's ops
cost the same as full-width ones, but the two phase-offset serial chains
interleave on the engines and the core becomes throughput-bound instead
of latency-bound.

PSUM: per pipe a double-buffered [128,1024] fp32 (2-bank) tile: rows 0:W
cols 0:768 hold the gates [f|o|g] (x-contribution pre-issued one step
ahead, h-matmuls accumulate on top); the 768:1024 pad takes the fp16
h-transposes of the tail. Warm-keeper junk matmuls (which keep the PE's
HAM activity window busy so the TensorE clock stays at full speed) write
into the next gates buffer right before its x pre-issue resets it.

x_dynamic is transposed on-chip via DMA-xbar transposes of [128,128] fp16
chunks (4 timesteps per chunk); timestep t lands at partition group
32*(t%4), so W_ih is replicated at the 4 partition bases; each pipe reads
its 64 batch columns of the chunk.
"""

import numpy as np

B_L = 128  # batch rows per core
NP = 2  # independent pipes per core
W = B_L // NP  # batch rows per pipe
T = 365
TP = 368  # T padded to a multiple of 4 for chunked transposes
DYN = 32
STATIC = 27
U = 256
NCORES = 8

_cached = {}


def _build_program(has_bias: bool):
    from contextlib import ExitStack

    import concourse.bacc as bacc
    import concourse.masks as masks
    import concourse.tile as tile
    from concourse import mybir

    f32 = mybir.dt.float32
    f16 = mybir.dt.float16
    AF = mybir.ActivationFunctionType

    nc = bacc.Bacc("TRN2", target_bir_lowering=False, debug=False)

    x_dyn = nc.dram_tensor("x_dynamic", [B_L, T * DYN], f32, kind="ExternalInput")
    x_st = nc.dram_tensor("x_static", [B_L, STATIC], f32, kind="ExternalInput")
    w_ih = nc.dram_tensor("weight_ih", [DYN, 3 * U], f32, kind="ExternalInput")
    w_hh = nc.dram_tensor("weight_hh", [U, 3 * U], f32, kind="ExternalInput")
    w_sh = nc.dram_tensor("weight_sh", [STATIC, U], f32, kind="ExternalInput")
    bias = nc.dram_tensor("bias", [1, 3 * U], f32, kind="ExternalInput")
    bias_s = nc.dram_tensor("bias_s", [1, U], f32, kind="ExternalInput")
    out = nc.dram_tensor("out", [B_L, U], f32, kind="ExternalOutput")

    # gates column layout inside the psum tile: [f | o | g | pad]
    F0, O0, G0, P0 = 0, U, 2 * U, 3 * U

    with tile.TileContext(nc) as tc, ExitStack() as ctx:
        const = ctx.enter_context(tc.tile_pool(name="const", bufs=1))
        xtiles = [
            const.tile([128, B_L], f16, tag=f"xt{c}", name=f"xt{c}")
            for c in range(TP // 4)
        ]
        Wih4 = const.tile([128, 3 * U], f16)  # W_ih replicated at 4 bases
        Whh0 = const.tile([128, 3 * U], f16)
        Whh1 = const.tile([128, 3 * U], f16)
        Wshb = const.tile([STATIC + 1, U], f16)  # rows 0-26 W_sh, row 27 bias_s
        xsT = const.tile([128, B_L], f16)
        ident = const.tile([128, 128], f16)
        igate = [
            const.tile([W, U], f16, tag=f"ig{p}", name=f"ig{p}") for p in range(NP)
        ]
        if has_bias:
            ones_row = const.tile([1, B_L], f16)
            bias16 = const.tile([1, 3 * U], f16)

        # one [128,1024] fp32 (2-bank) psum tile per pipe per step, 2 deep
        psum = ctx.enter_context(tc.tile_pool(name="ps", bufs=2, space="PSUM"))

        st = ctx.enter_context(tc.tile_pool(name="state", bufs=2))
        tmp = ctx.enter_context(tc.tile_pool(name="tmp", bufs=2))

        c_prev = [None] * NP
        hT = [[None, None] for _ in range(NP)]
        for p in range(NP):
            c_prev[p] = st.tile([W, U], f16, tag=f"c{p}", name=f"c{p}")
            nc.vector.memset(c_prev[p][:], 0.0)
            for half in (0, 1):
                hT[p][half] = st.tile(
                    [128, W], f16, tag=f"h{p}{half}", name=f"h{p}{half}"
                )
                nc.vector.memset(hT[p][half][:], 0.0)

        with tc.tile_pool(name="stage", bufs=1) as stage:
            wst = stage.tile([128, 3 * U], f32)
            nc.sync.dma_start(wst[:], w_hh[0:128, :])
            cpw = nc.vector.tensor_copy(Whh0[:], wst[:])
            nc.sync.dma_start(wst[:], w_hh[128:256, :])
            nc.vector.tensor_copy(Whh1[:], wst[:])
            wih32 = stage.tile([DYN, 3 * U], f32)
            nc.sync.dma_start(wih32[:], w_ih[:])
            for g in range(4):
                nc.vector.tensor_copy(Wih4[32 * g : 32 * g + 32, :], wih32[:])
            wsh32 = stage.tile([STATIC, U], f32)
            nc.sync.dma_start(wsh32[:], w_sh[:])
            nc.vector.tensor_copy(Wshb[0:STATIC, :], wsh32[:])
            bs32 = stage.tile([1, U], f32)
            nc.sync.dma_start(bs32[:], bias_s[:])
            bs16 = stage.tile([1, U], f16)
            nc.vector.tensor_copy(bs16[:], bs32[:])
            # partition 27 is not engine-addressable; DMA has no such limit
            nc.sync.dma_start(Wshb[STATIC : STATIC + 1, :], bs16[:])
            if has_bias:
                b32 = stage.tile([1, 3 * U], f32)
                nc.sync.dma_start(b32[:], bias[:])
                nc.vector.tensor_copy(bias16[:], b32[:])
                nc.vector.memset(ones_row[:], 1.0)

            # --- x_static -> transposed [27, 128] + ones row 27 ---
            xst32 = stage.tile([B_L, STATIC], f32)
            nc.sync.dma_start(xst32[:], x_st[:])
            xst16 = stage.tile([B_L, 128], f16)
            nc.vector.memset(xst16[:], 0.0)
            nc.vector.tensor_copy(xst16[:, 0:STATIC], xst32[:])
            nc.vector.memset(xst16[:, STATIC : STATIC + 1], 1.0)
            nc.sync.dma_start_transpose(xsT[:], xst16[:])

            masks.make_identity(nc, ident[:])

            # --- i_gate = sigmoid(x_static @ W_sh + bias_s), per pipe ---
            ig_ps = psum.tile([128, 4 * U], f32, tag="g0")
            for p in range(NP):
                nc.tensor.matmul(
                    ig_ps[0:W, p * U : (p + 1) * U],
                    xsT[0 : STATIC + 1, p * W : (p + 1) * W],
                    Wshb[:],
                    start=True,
                    stop=True,
                )
                nc.scalar.activation(
                    igate[p][:], ig_ps[0:W, p * U : (p + 1) * U], AF.Sigmoid
                )

            # --- x_dynamic: load fp32, convert fp16, transpose in chunks ---
            x16 = stage.tile([B_L, TP * DYN], f16)
            nc.vector.memset(x16[:, T * DYN :], 0.0)
            NCH = 4
            split = NCH * 128
            xs32a = stage.tile([B_L, split], f32)
            nc.sync.dma_start(xs32a[:], x_dyn[:, 0:split])
            nc.vector.tensor_copy(x16[:, 0:split], xs32a[:])
            for c in range(NCH):
                nc.sync.dma_start_transpose(
                    xtiles[c][:], x16[:, c * 128 : (c + 1) * 128]
                )
            xs32b = stage.tile([B_L, T * DYN - split], f32)
            nc.sync.dma_start(xs32b[:], x_dyn[:, split:])
            nc.vector.tensor_copy(x16[:, split : T * DYN], xs32b[:])
            for c in range(NCH, TP // 4):
                nc.sync.dma_start_transpose(
                    xtiles[c][:], x16[:, c * 128 : (c + 1) * 128]
                )

        def x_pre(p, t, ps):
            """x-contribution pre-issue for pipe p step t into gates tile."""
            g4 = 32 * (t % 4)
            cc = t // 4
            xt = xtiles[cc][g4 : g4 + 32, p * W : (p + 1) * W]
            wx = Wih4[g4 : g4 + 32, :]
            mms = [
                nc.tensor.matmul(
                    ps[0:W, 0 : 2 * U],
                    xt,
                    wx[:, 0 : 2 * U],
                    start=True,
                    stop=False,
                    tile_position=(g4, 0),
                ),
                nc.tensor.matmul(
                    ps[0:W, 2 * U : 3 * U],
                    xt,
                    wx[:, 2 * U : 3 * U],
                    start=True,
                    stop=False,
                    tile_position=(g4, 0),
                ),
            ]
            if has_bias:
                mms.append(
                    nc.tensor.matmul(
                        ps[0:W, 0 : 3 * U],
                        ones_row[:, p * W : (p + 1) * W],
                        bias16[:, 0 : 3 * U],
                        start=False,
                        stop=False,
                    )
                )
            return mms

        def warm_fill(after, ps_next, n512):
            """junk matmuls into the next gates buffer (its x pre-issue
            start=True reset follows in PE program order)"""
            prev = after
            for _ in range(n512):
                f = nc.tensor.matmul(
                    ps_next[0:W, 0:512],
                    Whh0[:, 128 : 128 + W],
                    Whh0[:, 0:512],
                    start=True,
                    stop=True,
                    skip_group_check=True,
                )
                tile.add_dep_helper(f.ins, prev.ins, sync=False, reason="warm order")
                prev = f
            return prev

        # acquire the fast PE clock once: the HAM needs ~3.4us of contiguous
        # matmul activity; a dense burst during input staging flips it
        junk = psum.tile([128, 4 * U], f32, tag="g1")
        prev = cpw
        for _ in range(45):
            f = nc.tensor.matmul(
                junk[0:128, 0:512],
                ident[:],
                Whh0[:, 0:512],
                start=True,
                stop=True,
                skip_group_check=True,
            )
            tile.add_dep_helper(f.ins, prev.ins, sync=False, reason="warm burst")
            prev = f

        # per-pipe rolling state across phases
        gates = [None] * NP  # current step's gates tile
        gates_n = [None] * NP  # next step's gates tile (x pre-issued)
        m1t = [None] * NP
        m2t = [None] * NP
        sot = [None] * NP
        c_new = [None] * NP

        for p in range(NP):
            ps = psum.tile([128, 4 * U], f32, tag=f"g{p}")
            x_pre(p, 0, ps)
            gates[p] = ps

        def head(p, t):
            """h-matmuls + gate activations + c-products for pipe p step t."""
            lo, hi = p * W, (p + 1) * W
            ps = gates[p]
            mms = [
                nc.tensor.matmul(
                    ps[0:W, F0:O0], hT[p][0][:], Whh0[:, 0:U], start=False, stop=False
                ),
                nc.tensor.matmul(
                    ps[0:W, F0:O0], hT[p][1][:], Whh1[:, 0:U], start=False, stop=False
                ),
                nc.tensor.matmul(
                    ps[0:W, G0:P0],
                    hT[p][0][:],
                    Whh0[:, 2 * U : 3 * U],
                    start=False,
                    stop=False,
                ),
                nc.tensor.matmul(
                    ps[0:W, G0:P0],
                    hT[p][1][:],
                    Whh1[:, 2 * U : 3 * U],
                    start=False,
                    stop=True,
                ),
                nc.tensor.matmul(
                    ps[0:W, O0:G0],
                    hT[p][0][:],
                    Whh0[:, U : 2 * U],
                    start=False,
                    stop=False,
                ),
                nc.tensor.matmul(
                    ps[0:W, O0:G0],
                    hT[p][1][:],
                    Whh1[:, U : 2 * U],
                    start=False,
                    stop=True,
                ),
            ]
            for a, b in zip(mms[1:], mms[:-1]):
                tile.add_dep_helper(a.ins, b.ins, sync=False, reason="mm order")
            anchor = mms[-1]
            # pre-issue x for t+1 into the next ring slot
            if t + 1 < T:
                ps_n = psum.tile([128, 4 * U], f32, tag=f"g{p}")
                anchor = warm_fill(anchor, ps_n, 3)
                xmm = x_pre(p, t + 1, ps_n)
                tile.add_dep_helper(
                    xmm[0].ins, anchor.ins, sync=False, reason="x after warm"
                )
                anchor = xmm[-1]
                gates_n[p] = ps_n
            else:
                gates_n[p] = None

            # gate activations: sigmoid(f) first, tanh(g), sigmoid(o) last
            sf = tmp.tile([W, U], f16, tag=f"sf{p}")
            af = nc.scalar.activation(sf[:], ps[0:W, F0:O0], AF.Sigmoid)
            tg = tmp.tile([W, U], f16, tag=f"tg{p}")
            ag = nc.scalar.activation(tg[:], ps[0:W, G0:P0], AF.Tanh)
            so = tmp.tile([W, U], f16, tag=f"so{p}")
            ao = nc.scalar.activation(so[:], ps[0:W, O0:G0], AF.Sigmoid)
            tile.add_dep_helper(ag.ins, af.ins, sync=False, reason="act order")
            tile.add_dep_helper(ao.ins, ag.ins, sync=False, reason="act order")
            sot[p] = so

            m1 = tmp.tile([W, U], f16, tag=f"m1{p}")
            m2 = tmp.tile([W, U], f16, tag=f"m2{p}")
            nc.vector.tensor_mul(m1[:], sf[:], c_prev[p][:])
            nc.vector.tensor_mul(m2[:], igate[p][:], tg[:])
            m1t[p], m2t[p] = m1, m2

        def tail(p, t):
            """c update + h production + transposed h for pipe p step t."""
            last = t == T - 1
            lo, hi = p * W, (p + 1) * W
            cn = st.tile([W, U], f16, tag=f"c{p}")
            nc.vector.tensor_add(cn[:], m1t[p][:], m2t[p][:])
            c_new[p] = cn
            if last:
                tch = tmp.tile([W, U], f32, tag=f"tc32{p}")
                nc.scalar.activation(tch[:], cn[:], AF.Tanh)
                h_out = tmp.tile([W, U], f32, tag=f"hout{p}")
                nc.vector.tensor_mul(h_out[:], sot[p][:], tch[:])
                nc.sync.dma_start(out[lo:hi, :], h_out[:])
            else:
                tch = tmp.tile([W, U], f16, tag=f"tc{p}")
                nc.scalar.activation(tch[:], cn[:], AF.Tanh)
                hh = tmp.tile([W, U], f16, tag=f"hh{p}")
                nc.vector.tensor_mul(hh[:], sot[p][:], tch[:])
                # transpose h into the gates tile's pad, then copy to SBUF
                pad = gates[p][:, P0 : P0 + 128].bitcast(f16)
                for half in (0, 1):
                    nc.tensor.matmul(
                        pad[:, 64 * half : 64 * half + W],
                        hh[:, 128 * half : 128 * half + 128],
                        ident[0:W, 0:W],
                        is_transpose=True,
                        start=True,
                        stop=True,
                        skip_group_check=True,
                    )
                for half in (0, 1):
                    ht_new = st.tile([128, W], f16, tag=f"h{p}{half}")
                    nc.vector.tensor_copy(
                        ht_new[:], pad[:, 64 * half : 64 * half + W]
                    )
                    hT[p][half] = ht_new
            c_prev[p] = c_new[p]
            gates[p] = gates_n[p]

        # phase-interleaved main loop: pipe 1 runs half a step behind pipe 0
        for t in range(T):
            head(0, t)
            if t > 0:
                tail(1, t - 1)
            head(1, t)
            tail(0, t)
        tail(1, T - 1)

    nc.compile()
    return nc


def get_program(has_bias: bool = False):
    if has_bias not in _cached:
        _cached[has_bias] = _build_program(has_bias)
    return _cached[has_bias]


def make_in_maps(inputs):
    x_dynamic = np.asarray(inputs["x_dynamic"], dtype=np.float32)
    x_static = np.asarray(inputs["x_static"], dtype=np.float32)
    w_ih = np.ascontiguousarray(np.asarray(inputs["weight_ih"], dtype=np.float32))
    w_hh = np.ascontiguousarray(np.asarray(inputs["weight_hh"], dtype=np.float32))
    w_sh = np.ascontiguousarray(np.asarray(inputs["weight_sh"], dtype=np.float32))
    bias = np.ascontiguousarray(
        np.asarray(inputs["bias"], dtype=np.float32).reshape(1, 3 * U)
    )
    bias_s = np.ascontiguousarray(
        np.asarray(inputs["bias_s"], dtype=np.float32).reshape(1, U)
    )
    in_maps = []
    for i in range(NCORES):
        sl = slice(i * B_L, (i + 1) * B_L)
        in_maps.append(
            {
                "x_dynamic": np.ascontiguousarray(
                    x_dynamic[sl].reshape(B_L, T * DYN)
                ),
                "x_static": np.ascontiguousarray(x_static[sl]),
                "weight_ih": w_ih,
                "weight_hh": w_hh,
                "weight_sh": w_sh,
                "bias": bias,
                "bias_s": bias_s,
            }
        )
    return in_maps


def kernel(**inputs) -> np.ndarray:
    from concourse.bass_utils import run_bass_kernel_spmd

    has_bias = bool(np.any(np.asarray(inputs["bias"])))
    nc = get_program(has_bias)
    in_maps = make_in_maps(inputs)
    res = run_bass_kernel_spmd(nc, in_maps, core_ids=list(range(NCORES)))
    return np.concatenate([r["out"] for r in res.results], axis=0).astype(np.float32)
